# revision 1
# baseline (speedup 1.0000x reference)
"""Trainium2 Bass kernel for nn_DSCAMSFF (1x1 conv + per-group CBAM gating).

Only x4 is live in the reference model (cov1-3 / the attention path are dead
code). Effective computation per batch b:

  a  = conv1x1(x4[b]) : [512, 256]          (w [512,2048], pixels flattened)
  per group g (channels of group g are a[(g%2)*256 : (g%2+1)*256]):
    avg_g = mean_px(a_g)                       [256]
    h_g   = relu(fc1_w[g] @ avg_g + fc1_b[g])  [64]
    ca_g  = sigmoid(fc2_w[g] @ h_g + fc2_b[g]) [256]
    sa_g  = sigmoid((ca_g*sa_w[g]) . a_g + sa_b[g])   [256 px]
    z_g   = sigmoid(a_g * ca_g[:,None] * sa_g[None,:])
    mean_g = mean(z_g)
    out_g = a_g * (1 + where(z_g > mean_g, 1, z_g))

Sharding: pure data-parallel over batch (8 cores x 1 batch element),
parameters replicated.

v3 implementation notes:
 - single s16 param DMA (dense fc2 weights, conv bias, one-hot replicate
   matrix all folded in); input stream order xa,w0,xb,w1,s16,w2,w3,s32.
 - fp16 output staged in [P,4,PX] SBUF chunks (2KB/partition stores).
 - per-half gating is emitted BETWEEN the conv halves so group 0/1 gating
   hides under the second conv half's DMA window (PE queues are in-order).
 - spatial attention: pre-activations for 4 groups land on psum partitions
   0..3 via one matmul pair; ACT sigmoids those 4 rows; a one-hot matmul
   replicates to 128 partitions; DVE evicts fp16.
 - t = a*sa via one fp16 tensor_tensor (2x DVE rate); ca folds into the
   z-sigmoid's per-partition scale; z means via DVE tensor_reduce (no ACT
   accumulator reads).
 - PE warmup + filler matmuls keep the tensor engine p-state ramped while
   DMA streams.
"""

import numpy as np

N_CORES = 8
P = 128
PX = 256            # 16*16 pixels
KT = 16             # 2048 / 128 K tiles
MT = 4              # 512 / 128 conv out tiles

# fp16 packed params (columns)
_E16_OFF = 0        # one-hot replicate lhsT [i, c] on partitions 0-3, 512
_W1_OFF = 512       # [p, kt, mm]  2*2*256 = 1024
_W2_OFF = 1536      # [p, i, s, m] with bias/ones rows 2*4*2*128 = 2048
_FB_OFF = 3584      # fc1 bias row0 [p, mt, m] 512
_CB_OFF = 4096      # conv bias row0 [m, mc] 512
_NSM16 = 4608
# fp32 packed params
_SAW_OFF = 0        # [p, s, i] 16
_SAB_OFF = 16       # col 16+4p, partition i holds sa_b[p+2i] (rows bias)
_B2_OFF = 24        # [p, s, i] 16
_SABR_OFF = 40      # sa_b[g] replicated all partitions, 8
_NSM32 = 48

_NWARM = 2          # PE warmup matmuls (free dim 512)
_NFILL1 = 4         # fillers after conv m0
_NFILL2 = 3         # fillers after evict0

_CACHE = {}


def _register_dve_ops():
    """Register the fused mask DVE op (idempotent, runtime-only)."""
    from concourse import dve_ops as DO
    from concourse.dve_spec import Spec, Src0, Src1, C0, One, select, lower
    from concourse.dve_uop import DveOpSpec

    if "DSCAM_MASK_MUL" in DO._SUB_OPCODE_FOR_NAME:
        by = {o.name: o for o in DO.OPS}
        return by["DSCAM_MASK_MUL"]

    def mk(name, spec):
        row = DO._CUSTOM_DVE_ROW_BASE + len(DO.OPS)
        DO._SUB_OPCODE_FOR_NAME[name] = row
        shas = {}
        for ver in ("v3", "v4"):
            try:
                uops = lower(spec, ver=ver)
                shas[ver] = DveOpSpec(name=name, opcode=row, uops=uops,
                                      rd1_en=True).sha(ver)
            except Exception:
                pass
        op = DO.DveOp(name, spec, subdim=False, uops_sha=shas)
        DO.OPS.append(op)
        DO.CUSTOM_DVE_SPECS[name] = spec
        return op

    msk = mk("DSCAM_MASK_MUL", Spec(
        body=Src1 * (One + select(Src0 > C0, One, Src0)),
        reference=lambda in0, in1, s0, s1, imm2:
            (in1.astype(np.float32)
             * (1.0 + np.where(in0.astype(np.float32) > s0, 1.0,
                               in0.astype(np.float32)))).astype(np.float32),
    ))
    return msk


def _build_program():
    import concourse.mybir as mybir
    import concourse.tile as tile
    from concourse import bacc

    fp32 = mybir.dt.float32
    fp16 = mybir.dt.float16
    Act = mybir.ActivationFunctionType
    Alu = mybir.AluOpType
    AX = mybir.AxisListType

    _MSK_OP = _register_dve_ops()

    nc = bacc.Bacc("TRN2", target_bir_lowering=False, debug=False)

    x_d = nc.dram_tensor("x", [P, KT, PX], fp16, kind="ExternalInput").ap()
    w_d = nc.dram_tensor("w", [MT, P, KT, P], fp16, kind="ExternalInput").ap()
    s16_d = nc.dram_tensor("s16", [P, _NSM16], fp16, kind="ExternalInput").ap()
    p0_d = nc.dram_tensor("p0", [P, 64], fp16, kind="ExternalInput").ap()
    out_d = nc.dram_tensor("out", [P, 16, PX], fp16, kind="ExternalOutput").ap()

    with tile.TileContext(nc) as tc:
        with (
            tc.tile_pool(name="singles", bufs=1) as singles,
            tc.tile_pool(name="work", bufs=8) as work,
            tc.tile_pool(name="tpool", bufs=4) as tpool,
            tc.tile_pool(name="zpool", bufs=6) as zpool,
            tc.tile_pool(name="otp", bufs=4) as otp,
            tc.tile_pool(name="psA", bufs=2, space="PSUM") as psA,
            tc.tile_pool(name="psR", bufs=3, space="PSUM") as psR,
            tc.tile_pool(name="psB", bufs=2, space="PSUM") as psB,
        ):
            # ---- input tiles ----
            xa = singles.tile([P, 8, PX], fp16, tag="xa")
            xb = singles.tile([P, 8, PX], fp16, tag="xb")
            wt = [None] * MT
            for m in range(MT):
                wt[m] = singles.tile([P, KT, P], fp16, tag=f"w{m}",
                                     name=f"w{m}")
            s16 = singles.tile([P, _NSM16], fp16, tag="s16")
            p0 = singles.tile([P, 64], fp16, tag="p0")

            # ---- input DMAs on both hwdge rings, ordered by first need ----
            nc.sync.dma_start(out=p0, in_=p0_d)
            nc.sync.dma_start(out=xa, in_=x_d[:, :8, :])
            nc.scalar.dma_start(out=wt[0], in_=w_d[0])
            nc.sync.dma_start(out=xb, in_=x_d[:, 8:, :])
            nc.scalar.dma_start(out=wt[1], in_=w_d[1])
            nc.sync.dma_start(out=s16, in_=s16_d)
            nc.scalar.dma_start(out=wt[2], in_=w_d[2])
            nc.sync.dma_start(out=wt[3], in_=w_d[3])

            # parameter views
            e16v = s16[:, _E16_OFF:_E16_OFF + 512].rearrange(
                "P (i c) -> P i c", i=4)
            w1v = s16[:, _W1_OFF:_W1_OFF + 1024].rearrange(
                "P (p k m) -> P p k m", p=2, k=2)
            w2v = s16[:, _W2_OFF:_W2_OFF + 2048].rearrange(
                "P (p i s m) -> P p i s m", p=2, i=4, s=2)
            fbias = s16[0:1, _FB_OFF:_FB_OFF + 512].rearrange(
                "o (p t c) -> o p t c", p=2, t=2)
            cbias = s16[0:1, _CB_OFF:_CB_OFF + 512].rearrange(
                "o (m c) -> o m c", m=4)
            cbv = p0[:, 0:4]                 # partition c: cov4_b[m*128+c]
            sawv = p0[:, 4:20].rearrange(
                "P (p s i) -> P p s i", p=2, s=2)
            sabv = p0[:, 20:28]              # col 20+4p, partition i
            b2v = p0[:, 28:44].rearrange(
                "P (p s i) -> P p s i", p=2, s=2)

            # constants
            onesPK = singles.tile([P, 512], fp16, tag="onesPK")
            nc.gpsimd.memset(onesPK, 1.0)
            ones2 = singles.tile([1, PX], fp16, tag="ones2")
            nc.gpsimd.memset(ones2, 1.0)
            ones32 = singles.tile([P, P], fp32, tag="ones32")
            nc.gpsimd.memset(ones32, 1.0)

            hm = [singles.tile([P, 4], fp16, tag="hm0", name="hm0"),
                  singles.tile([P, 4], fp16, tag="hm1", name="hm1")]
            nc.gpsimd.memset(hm[0], 0.0)
            nc.gpsimd.memset(hm[1], 0.0)

            # ACT table preload + PE p-state warmup while inputs stream
            tl = singles.tile([1, 1], fp32, tag="tl")
            nc.scalar.activation(out=tl, in_=ones2[:, 0:1], func=Act.Sigmoid)

            def fill(n, free=512):
                for _ in range(n):
                    wps = psA.tile([P, 512], fp32, tag="conv", name="wps")
                    nc.tensor.matmul(wps[:, 0:free], lhsT=onesPK[:, 0:P],
                                     rhs=onesPK[:, 0:free],
                                     start=True, stop=True)

            fill(_NWARM)

            a16 = [None, None]
            asum16 = [None, None]
            h_sb = [None, None]
            ca = [None, None]
            weff16 = [None, None]
            zsum = [None, None]
            pm = [None, None]

            def conv_m(p, s, ps):
                m = 2 * p + s
                for kt in range(KT):
                    xsrc = xa if kt < 8 else xb
                    nc.tensor.matmul(
                        ps[s], lhsT=wt[m][:, kt, :],
                        rhs=xsrc[:, kt % 8, :],
                        start=(kt == 0), stop=(kt == KT - 1))

            def evict_p(p, ps):
                a16[p] = singles.tile([P, 2, PX], fp16, tag=f"a16_{p}",
                                      name=f"a16_{p}")
                asum = work.tile([P, 2], fp32, tag="asum")
                for s in (0, 1):
                    nc.vector.tensor_copy(out=a16[p][:, s, :], in_=ps[s])
                    nc.vector.tensor_reduce(asum[:, s:s + 1], ps[s],
                                            axis=AX.X, op=Alu.add)
                asum16[p] = singles.tile([P, 2], fp16, tag=f"as16_{p}",
                                         name=f"as16_{p}")
                # 1/256 pixel-mean folded here (not into fp16 weights)
                nc.vector.tensor_scalar_mul(asum16[p], asum, 1.0 / 256.0)

            def fc_chain(p):
                # fc1: h = relu(W1 @ avg + b1), 4 groups of 64 stacked
                hp = psB.tile([P, 2], fp32, tag="tiny8")
                for mt in (0, 1):
                    for kt in (0, 1):
                        nc.tensor.matmul(
                            hp[:, mt:mt + 1],
                            lhsT=w1v[:, p, kt, mt * P:(mt + 1) * P],
                            rhs=asum16[p][:, kt:kt + 1],
                            start=(kt == 0), stop=False)
                    nc.tensor.matmul(
                        hp[:, mt:mt + 1], lhsT=fbias[:, p, mt, :],
                        rhs=ones2[:, 0:1], start=False, stop=True)
                h_sb[p] = singles.tile([P, 2], fp16, tag=f"h{p}", name=f"h{p}")
                nc.vector.tensor_scalar(
                    out=h_sb[p], in0=hp, scalar1=0.0, scalar2=None,
                    op0=Alu.max)
                # pack group h columns (evens rows 0-63, odds 64-127) into
                # the pre-zeroed h_m with two strided copies
                h_m = hm[p]
                nc.vector.tensor_copy(
                    out=h_m[0:64, 0:3:2], in_=h_sb[p][0:64, 0:2])
                nc.vector.tensor_copy(
                    out=h_m[64:128, 1:4:2], in_=h_sb[p][64:128, 0:2])
                cp = psB.tile([P, 2, 4], fp32, tag="tiny8")
                for s in (0, 1):
                    for i in range(4):
                        nc.tensor.matmul(
                            cp[:, s, i:i + 1], lhsT=w2v[:, p, i, s, :],
                            rhs=h_m[:, i:i + 1], start=True, stop=True)
                cab = work.tile([P, 2, 4], fp32, tag="cab")
                nc.vector.tensor_tensor(out=cab, in0=cp, in1=b2v[:, p],
                                        op=Alu.add)
                ca[p] = singles.tile([P, 2, 4], fp32, tag=f"ca{p}",
                                     name=f"ca{p}")
                nc.scalar.activation(out=ca[p], in_=cab, func=Act.Sigmoid)
                weff16[p] = singles.tile([P, 2, 4], fp16, tag=f"we{p}",
                                         name=f"we{p}")
                nc.vector.tensor_tensor(out=weff16[p], in0=ca[p],
                                        in1=sawv[:, p], op=Alu.mult)
                zsum[p] = singles.tile([P, 4, 2], fp32, tag=f"zs{p}",
                                       name=f"zs{p}")
                pm[p] = singles.tile([P, 4], fp32, tag=f"pm{p}",
                                     name=f"pm{p}")

            def sa_rows(p):
                # spatial-attention pre-activations for all 4 groups land on
                # psum partitions 0..3; DVE evicts the 4 rows (pre-sigmoid)
                srps = psB.tile([4, PX], fp32, tag="tiny8", name=f"srps{p}")
                for s in (0, 1):
                    nc.tensor.matmul(srps, lhsT=weff16[p][:, s, :],
                                     rhs=a16[p][:, s, :],
                                     start=(s == 0), stop=(s == 1))
                srow = work.tile([4, PX], fp16, tag="srow")
                nc.scalar.activation(out=srow, in_=srps, func=Act.Sigmoid,
                                     bias=sabv[0:4, 4 * p:4 * p + 1])
                return srow

            def srep_mm(p, i, srow):
                # replicate group i's pre-sigmoid sa row to 128 partitions
                srep = psR.tile([P, PX], fp32, tag="srep", name=f"sr{p}{i}")
                nc.tensor.matmul(srep, lhsT=e16v[0:4, i, :], rhs=srow,
                                 start=True, stop=True)
                return srep

            def group_t(p, i, srep):
                # DVE evicts the replicated (already sigmoided) sa row and
                # multiplies t = a*sa back-to-back on the same engine
                srep16 = work.tile([P, PX], fp16, tag="srep16")
                nc.vector.tensor_copy(out=srep16, in_=srep)
                t16 = tpool.tile([P, 2, PX], fp16, tag="t16")
                nc.vector.tensor_tensor(
                    out=t16, in0=a16[p],
                    in1=srep16[:, None, :].to_broadcast((P, 2, PX)),
                    op=Alu.mult)
                return t16

            def group_z(p, i, t16):
                # z = sigmoid(ca*t) per half, group pixel-sums in the ACT
                # accumulators
                z = zpool.tile([P, 2, PX], fp16, tag="z")
                for s in (0, 1):
                    nc.scalar.activation(out=z[:, s, :], in_=t16[:, s, :],
                                         func=Act.Sigmoid,
                                         scale=ca[p][:, s, i:i + 1],
                                         accum_out=zsum[p][:, i, s:s + 1])
                return z

            def zr_mm(p, i, zr):
                nc.tensor.matmul(zr[:, i, :], lhsT=ones32,
                                 rhs=zsum[p][:, i, :],
                                 start=True, stop=True)

            def group_back(p, i, z, ot, zr):
                # fused out = a * (1 + where(z > mean, 1, z))
                nc.vector.tensor_scalar(
                    out=pm[p][:, i:i + 1], in0=zr[:, i, 0:1],
                    scalar1=zr[:, i, 1:2], scalar2=1.0 / 65536.0,
                    op0=Alu.add, op1=Alu.mult)
                nc.vector._custom_dve(
                    _MSK_OP, out=ot[:, 2 * (i % 2):2 * (i % 2) + 2, :],
                    in0=z, in1=a16[p], s0=pm[p][:, i:i + 1])

            # ---- schedule: gate p=0 interleaves with the second conv pair
            # at m-tile granularity; fillers bridge PE stalls so the tensor
            # engine p-state stays ramped ----
            ps0 = [psA.tile([P, PX], fp32, tag="conv", name=f"cv0{s}")
                   for s in (0, 1)]
            ps1 = [psA.tile([P, PX], fp32, tag="conv", name=f"cv1{s}")
                   for s in (0, 1)]
            # PE stream: m00,m01,m10 back-to-back (no gaps -> p-state holds),
            # fc0 + sa0 setup while w3 streams, then m11
            conv_m(0, 0, ps0)
            conv_m(0, 1, ps0)
            evict_p(0, ps0)
            conv_m(1, 0, ps1)
            fc_chain(0)
            srow0 = sa_rows(0)
            sreps0 = [srep_mm(0, 0, srow0), srep_mm(0, 1, srow0)]
            zs0 = [None] * 4
            ts0 = [None] * 4
            ots0 = [otp.tile([P, 4, PX], fp16, tag="ot", name="ot00"),
                    otp.tile([P, 4, PX], fp16, tag="ot", name="ot02")]
            ts0[0] = group_t(0, 0, sreps0[0])
            zs0[0] = group_z(0, 0, ts0[0])
            ts0[1] = group_t(0, 1, sreps0[1])
            conv_m(1, 1, ps1)
            zs0[1] = group_z(0, 1, ts0[1])
            sreps0 += [srep_mm(0, 2, srow0), srep_mm(0, 3, srow0)]
            ts0[2] = group_t(0, 2, sreps0[2])
            zs0[2] = group_z(0, 2, ts0[2])
            evict_p(1, ps1)
            fc_chain(1)
            ts0[3] = group_t(0, 3, sreps0[3])
            zs0[3] = group_z(0, 3, ts0[3])
            # gate1 front-end prep rides between gate0's z calls on ACT
            srow1 = sa_rows(1)
            zr0 = psR.tile([P, 4, 2], fp32, tag="srep", name="zr0")
            for i in range(4):
                zr_mm(0, i, zr0)
            sreps1 = [srep_mm(1, i, srow1) for i in range(4)]
            zs1 = [None] * 4
            ts1 = [None] * 4
            ts1[0] = group_t(1, 0, sreps1[0])
            ts1[1] = group_t(1, 1, sreps1[1])
            for i in range(4):
                group_back(0, i, zs0[i], ots0[i // 2], zr0)
                if i == 1:
                    nc.sync.dma_start(out=out_d[:, 0:4, :], in_=ots0[0])
                if i == 3:
                    nc.sync.dma_start(out=out_d[:, 4:8, :], in_=ots0[1])
            ots1 = [otp.tile([P, 4, PX], fp16, tag="ot", name="ot10"),
                    otp.tile([P, 4, PX], fp16, tag="ot", name="ot12")]
            zr1 = psR.tile([P, 4, 2], fp32, tag="srep", name="zr1")
            zs1[0] = group_z(1, 0, ts1[0])
            ts1[2] = group_t(1, 2, sreps1[2])
            zs1[1] = group_z(1, 1, ts1[1])
            zr_mm(1, 0, zr1)
            ts1[3] = group_t(1, 3, sreps1[3])
            group_back(1, 0, zs1[0], ots1[0], zr1)
            zs1[2] = group_z(1, 2, ts1[2])
            zr_mm(1, 1, zr1)
            group_back(1, 1, zs1[1], ots1[0], zr1)
            nc.sync.dma_start(out=out_d[:, 8:12, :], in_=ots1[0])

            zs1[3] = group_z(1, 3, ts1[3])
            zr_mm(1, 2, zr1)
            group_back(1, 2, zs1[2], ots1[1], zr1)
            nc.sync.dma_start(out=out_d[:, 12:14, :], in_=ots1[1][:, 0:2, :])
            zr_mm(1, 3, zr1)
            group_back(1, 3, zs1[3], ots1[1], zr1)
            nc.sync.dma_start(out=out_d[:, 14:16, :], in_=ots1[1][:, 2:4, :])

    nc.finalize()
    return nc


def _prep_core_inputs(x4b, w, s16, p0):
    x = np.ascontiguousarray(
        x4b.reshape(KT, P, PX).transpose(1, 0, 2)).astype(np.float16)
    return {"x": x, "w": w, "s16": s16, "p0": p0}


def _prep_params(cov4_w, cov4_b, fc1_w, fc1_b, fc2_w, fc2_b, sa_w, sa_b):
    f32 = np.float32
    w2d = np.asarray(cov4_w, f32).reshape(512, 2048)
    wr = w2d.reshape(MT, P, KT, P)                  # [m, mc, kt, part]
    w_arr = np.ascontiguousarray(wr.transpose(0, 3, 2, 1)).astype(np.float16)

    fc1_w = np.asarray(fc1_w, f32)
    fc1_b = np.asarray(fc1_b, f32)
    fc2_w = np.asarray(fc2_w, f32)
    fc2_b = np.asarray(fc2_b, f32)
    sa_w = np.asarray(sa_w, f32)
    sa_b = np.asarray(sa_b, f32)

    w1 = np.zeros((P, 2, 2, 256), f32)
    w2 = np.zeros((P, 2, 4, 2, P), f32)
    b1 = np.zeros((2, 2, P), f32)
    saw = np.zeros((P, 2, 2, 4), f32)
    b2t = np.zeros((P, 2, 2, 4), f32)
    for p in range(2):
        W1s = np.concatenate([fc1_w[p + 2 * i] for i in range(4)], axis=0)
        b1s = np.concatenate([fc1_b[p + 2 * i] for i in range(4)], axis=0)
        for kt in range(2):
            w1[:, p, kt, :] = W1s[:, kt * P:(kt + 1) * P].T
        b1[p, 0] = b1s[:P]
        b1[p, 1] = b1s[P:]
        for i in range(4):
            g = p + 2 * i
            lo = 64 * (i % 2)
            brow = 64 if i % 2 == 0 else 0
            for s in range(2):
                w2[lo:lo + 64, p, i, s, :] = fc2_w[g][s * P:(s + 1) * P, :].T
                w2[brow, p, i, s, :] = fc2_b[g, s * P:(s + 1) * P]
                saw[:, p, s, i] = sa_w[g, s * P:(s + 1) * P]
                b2t[:, p, s, i] = fc2_b[g, s * P:(s + 1) * P]

    s16 = np.zeros((P, _NSM16), np.float16)
    # one-hot replicate lhsT: partition k (k<4), block i is 1 iff i==k
    for k in range(4):
        s16[k, _E16_OFF + k * P:_E16_OFF + (k + 1) * P] = 1.0
    s16[:, _W1_OFF:_W1_OFF + 1024] = w1.reshape(P, 1024).astype(np.float16)
    s16[:, _W2_OFF:_W2_OFF + 2048] = w2.reshape(P, 2048).astype(np.float16)
    s16[0, _FB_OFF:_FB_OFF + 512] = b1.reshape(512).astype(np.float16)
    s16[0, _CB_OFF:_CB_OFF + 512] = \
        np.asarray(cov4_b, f32).astype(np.float16).reshape(512)

    p0 = np.zeros((P, 64), np.float16)
    p0[:, 0:4] = np.asarray(cov4_b, f32).reshape(4, P).T.astype(np.float16)
    p0[:, 4:20] = saw.reshape(P, 16).astype(np.float16)
    # sa bias for the 4-row sigmoid: partition i holds sa_b[p+2i] at col 20+4p
    for p in range(2):
        for i in range(4):
            p0[i, 20 + 4 * p] = sa_b[p + 2 * i]
    p0[:, 28:44] = b2t.reshape(P, 16).astype(np.float16)
    return w_arr, s16, p0


def kernel(**inputs):
    from concourse.bass_utils import run_bass_kernel_spmd

    if "nc" not in _CACHE:
        _CACHE["nc"] = _build_program()
    nc = _CACHE["nc"]

    x4 = np.asarray(inputs["x4"], np.float32)
    B = x4.shape[0]
    w_arr, s16, p0 = _prep_params(
        inputs["cov4_w"], inputs["cov4_b"],
        inputs["gce_fc1_w"], inputs["gce_fc1_b"],
        inputs["gce_fc2_w"], inputs["gce_fc2_b"],
        inputs["gce_sa_w"], inputs["gce_sa_b"])

    in_maps = [
        _prep_core_inputs(x4[b].reshape(2048, PX), w_arr, s16, p0)
        for b in range(B)
    ]
    res = run_bass_kernel_spmd(nc, in_maps, list(range(N_CORES)))
    _CACHE["last_results"] = res

    out = np.empty((B, 2048, 16, 16), np.float32)
    for b in range(B):
        # out_d[part, 8p+2i+s, px] -> channel 512i+256p+128s+part
        arr = res.results[b]["out"].astype(np.float32)
        arr5 = arr.reshape(P, 2, 4, 2, PX)          # [part, p, i, s, px]
        out[b] = arr5.transpose(2, 1, 3, 0, 4).reshape(2048, 16, 16)
    return out

